# revision 21
# baseline (speedup 1.0000x reference)
"""CQAttention Trainium2 kernel.

Math (per batch b):
  S = (C*w3) @ Q^T + (C@w1)[:,None] + (Q@w2)[None,:] (+bias, dropped: softmax-invariant)
  Sq = softmax over q of qmask-masked S ; Sc = softmax over c of cmask-masked S
  A = Sq@Q ; Bm = Sq @ (Sc^T @ C) ; out = [C | A | C*A | C*Bm]

Key identity: ONE weight matrix E = exp(S + rq + qneg + rc + cneg) serves
BOTH softmaxes, because softmax-over-q cancels any per-c factor and
softmax-over-c cancels any per-q factor. Masks use a finite -30 (not
-1e30) so the cancellation stays finite: leakage weight e^-30 ~ 1e-13 is
far below the 2e-2 tolerance, and masked rows normalize exactly.

Device algorithm (all PE operands bf16, fp32 PSUM):
  CTb  = Cb^T                  (PE transposes of bf16 C tiles)
  ST   = QT3 @ CTb + 1x(rc+cneg)[c]   (rank-1 bias via K=1 matmul row)
  E    = exp(ST + (rq+qneg)[q])       (ACT bias; [q, c] bf16)
  ET   = PE transpose of E            ([c, q] bf16, raw PSUM->SBUF copy)
  t1   = sum_k ET_k^T @ [C|1]_k       [q, d+2] = unnorm Sc^T C | colsum
  T1s  = t1 * 1/colsum                [q, d] bf16 (per-q factors cancel)
  psA  = E_k^T @ [Q|1]                [c, d+1] = unnorm A | rowsum
  rr   = 1/rowsum ; A = psA*rr        (per-c factors cancel)
  CA   = A * Cb ; psB = E_k^T @ T1s ; CBm = (psB*rr)*Cb
Device output is bf16 [A | CA | CBm]; the C column of the final output
is the input itself and is concatenated on the host.

Sharding: data-parallel over batch, 4 batches per core on 8 cores.
"""

import numpy as np

MASK_NEG = -30.0
B_FULL, LC, LQ, D = 32, 1024, 128, 256
N_CORES = 8
NB = B_FULL // N_CORES  # batches per core
KC = LC // 128  # c-tiles per batch (8)

_CACHE = {}


def _build_nc():
    import concourse.bacc as bacc
    import concourse.mybir as mybir
    from concourse import tile
    from concourse.masks import make_identity

    fp32 = mybir.dt.float32
    bf16 = mybir.dt.bfloat16
    MULT = mybir.AluOpType.mult
    EXP = mybir.ActivationFunctionType.Exp

    nc = bacc.Bacc("TRN2", target_bir_lowering=False, debug=False)

    # bundle (bf16, per partition): Cb [KC*258] | QT3 [2*128] | Qb [257]
    #                               | pad | rq [1] | pad
    NBND = KC * (D + 2) + 2 * 128 + (D + 1) + 3
    bnd_d = nc.dram_tensor("bnd", [NB, 128, NBND], bf16, kind="ExternalInput")
    # rc rows on one partition: cols (b*2+h)*512.. hold (rc+cneg) halves
    rc_d = nc.dram_tensor("rc", [1, NB * 2 * 512], bf16, kind="ExternalInput")
    out_d = nc.dram_tensor("out", [NB, LC, 3 * D], bf16, kind="ExternalOutput")

    O_CB = 0
    O_QT = KC * (D + 2)  # 2064
    O_QB = O_QT + 2 * 128  # 2320
    O_RQ = O_QB + (D + 1) + 1  # 2578 (4-byte aligned)

    with tile.TileContext(nc) as tc:
        with (
            tc.tile_pool(name="const", bufs=1) as const,
            tc.tile_pool(name="cbpool", bufs=NB) as p_cb,
            tc.tile_pool(name="ctpool", bufs=2) as p_ct,
            tc.tile_pool(name="epool", bufs=2) as p_e,
            tc.tile_pool(name="etpool", bufs=2) as p_et,
            tc.tile_pool(name="opool", bufs=4) as p_o,
            tc.tile_pool(name="smpool", bufs=4) as p_sm,
            tc.tile_pool(name="pstr", bufs=2, space="PSUM") as ps_tr,
            tc.tile_pool(name="psst", bufs=1, space="PSUM") as ps_st,
            tc.tile_pool(name="pst1", bufs=1, space="PSUM") as ps_t1,
            tc.tile_pool(name="psab", bufs=4, space="PSUM") as ps_ab,
        ):
            ones1 = const.tile([1, 128], bf16)
            nc.gpsimd.memset(ones1, 1.0)
            identb = const.tile([128, 128], bf16)
            make_identity(nc, identb)
            # rc rows for all batches in one small load
            rc2 = const.tile([1, NB * 2 * 512], bf16)
            nc.gpsimd.dma_start(rc2, rc_d.ap())

            # PE p-state warmup: TRN2's tensor engine only reaches full
            # clock after ~3us of CONTINUOUS execution, and any idle gap
            # resets the ramp. Run dummy transposes while the first input
            # bundle loads so real matmuls start at full speed.
            warm = ps_tr.tile([128, 128], bf16, tag="tr", name="warm")
            for _ in range(32):
                nc.tensor.transpose(warm, identb, identb)

            # ---- hoisted input loads: one bf16 bundle per batch ----
            Cb1s, QT3s, Qb1s, rqfs = [], [], [], []
            for b in range(NB):
                bnd = p_cb.tile([128, NBND], bf16, tag="bnd")
                ldq = nc.sync if b % 2 == 0 else nc.scalar
                if b == 0:
                    # split across both rings so the first transposes
                    # start as early as possible
                    half = O_QT // 2
                    nc.sync.dma_start(bnd[:, 0:half], bnd_d.ap()[b, :, 0:half])
                    nc.scalar.dma_start(
                        bnd[:, half:NBND], bnd_d.ap()[b, :, half:NBND]
                    )
                else:
                    ldq.dma_start(bnd, bnd_d.ap()[b])
                Cb1s.append(
                    bnd[:, O_CB:O_QT].rearrange("p (k d) -> p k d", d=D + 2)
                )
                QT3s.append(bnd[:, O_QT:O_QB].rearrange("p (t d) -> p t d", d=128))
                Qb1s.append(bnd[:, O_QB : O_QB + D + 1])
                # fp32 copy of the exp bias (ACT bias wants fp32)
                rqf = p_sm.tile([128, 1], fp32, tag="rqf", name=f"rqf{b}")
                nc.gpsimd.tensor_copy(rqf, bnd[:, O_RQ : O_RQ + 1])
                rqfs.append(rqf)

            Ex, T1sx, osbx, rrx = {}, {}, {}, {}

            def head_stages(b):
                """Head of batch b (incl. the T1s-independent psA chain)."""
                Cb1, QT3, Qb1 = Cb1s[b], QT3s[b], Qb1s[b]
                rqf = rqfs[b]
                CTb = p_ct.tile([128, 2, LC], bf16, tag="ct", name=f"CTb{b}")
                E = p_e.tile([128, LC], bf16, tag="e", name=f"E{b}")
                Ex[b] = E
                ET = p_et.tile([128, KC, 128], bf16, tag="et", name=f"ET{b}")

                def ct_group(g):
                    dk, h = g // 2, g % 2
                    pt = ps_tr.tile([128, 512], bf16, tag="tr", name=f"pt{b}_{g}")
                    for j in range(4):
                        k = h * 4 + j
                        nc.tensor.transpose(
                            pt[:, j * 128 : (j + 1) * 128],
                            Cb1[:, k, dk * 128 : (dk + 1) * 128],
                            identb,
                        )
                    # PSUM->SBUF copies split DVE/ACT (GPSIMD can't see PSUM)
                    if g % 2 == 0:
                        nc.vector.tensor_copy(
                            CTb[:, dk, h * 512 : (h + 1) * 512], pt
                        )
                    else:
                        nc.scalar.mul(CTb[:, dk, h * 512 : (h + 1) * 512], pt, 1.0)

                def st_half(h):
                    st = ps_st.tile([128, 512], fp32, tag="st", name=f"st{b}_{h}")
                    nc.tensor.matmul(
                        st, QT3[:, 0], CTb[:, 0, h * 512 : (h + 1) * 512],
                        start=True, stop=False,
                    )
                    nc.tensor.matmul(
                        st, QT3[:, 1], CTb[:, 1, h * 512 : (h + 1) * 512],
                        start=False, stop=False,
                    )
                    # rank-1 row: adds (rc+cneg)[c] to every q row
                    nc.tensor.matmul(
                        st, ones1,
                        rc2[0:1, (2 * b + h) * 512 : (2 * b + h + 1) * 512],
                        start=False, stop=True,
                    )
                    nc.scalar.activation(
                        E[:, h * 512 : (h + 1) * 512], st, EXP, bias=rqf
                    )

                def et_group(g):
                    # group g covers E tiles 4g..4g+3 (group 0 only needs
                    # the first exp half -> fills the exp(h1) latency)
                    pe = ps_tr.tile([128, 512], bf16, tag="tr", name=f"pe{b}_{g}")
                    for j in range(4):
                        k = g * 4 + j
                        nc.tensor.transpose(
                            pe[:, j * 128 : (j + 1) * 128],
                            E[:, k * 128 : (k + 1) * 128],
                            identb,
                        )
                    nc.vector.tensor_copy(ET[:, 4 * g : 4 * g + 4], pe)

                def tile_a(k):
                    kk = k % 4
                    if kk == 0:
                        osbx[(b, k // 4)] = p_o.tile(
                            [128, 4, 3 * D], bf16, tag="osb", name=f"osb{b}_{k}"
                        )
                    osb = osbx[(b, k // 4)]
                    e_k = E[:, k * 128 : (k + 1) * 128]
                    # psA = E_k^T @ [Q|1]: rowsum in col D
                    psA = ps_ab.tile([128, D + 1], fp32, tag="ab", name=f"psA{b}_{k}")
                    nc.tensor.matmul(psA, e_k, Qb1, start=True, stop=True)
                    rr = p_sm.tile(
                        [128, 1], fp32, tag="rr", name=f"rr{b}_{k}", bufs=18
                    )
                    rrx[(b, k)] = rr
                    nc.vector.reciprocal(rr, psA[:, D : D + 1])
                    # A = psA * rr (per-partition scale; 2 of 8 on DVE)
                    if k in (1, 5):
                        nc.vector.tensor_scalar_mul(osb[:, kk, 0:D], psA[:, 0:D], rr)
                    else:
                        nc.scalar.mul(osb[:, kk, 0:D], psA[:, 0:D], rr)
                    if kk == 3:
                        # CA = C * A for 4 tiles in one GPSIMD op
                        nc.gpsimd.tensor_mul(
                            osb[:, :, D : 2 * D],
                            Cb1[:, k - 3 : k + 1, 0:D],
                            osb[:, :, 0:D],
                        )

                t1_box = {}

                def t1_acc(half):
                    if half == 0:
                        t1_box["t1"] = ps_t1.tile(
                            [128, D + 2], fp32, tag="t1", name=f"t1_{b}"
                        )
                    t1 = t1_box["t1"]
                    for k in range(4 * half, 4 * half + 4):
                        nc.tensor.matmul(
                            t1,
                            ET[:, k],
                            Cb1[:, k],
                            start=(k == 0),
                            stop=(k == KC - 1),
                        )
                    if half == 1:
                        recipT = p_sm.tile(
                            [128, 1], fp32, tag="recipT", name=f"rT{b}"
                        )
                        nc.vector.reciprocal(recipT, t1[:, D : D + 1])
                        T1s = p_sm.tile([128, D], bf16, tag="t1s", name=f"T1s{b}")
                        nc.scalar.mul(T1s, t1[:, 0:D], recipT)
                        T1sx[b] = T1s

                return [
                    lambda: ct_group(0),
                    lambda: ct_group(1),
                    lambda: ct_group(2),
                    lambda: ct_group(3),
                    lambda: st_half(0),
                    lambda: st_half(1),
                    lambda: et_group(0),
                    lambda: (tile_a(0), tile_a(1), et_group(1)),
                    lambda: (t1_acc(0), tile_a(2), tile_a(3)),
                    lambda: (t1_acc(1), tile_a(4), tile_a(5)),
                    lambda: (tile_a(6), tile_a(7)),
                ]

            def tail_stages(b):
                """T1s-dependent part of batch b's tail: psB / CBm."""
                E = Ex[b]
                Cb1 = Cb1s[b]

                def tile_b(k):
                    T1s = T1sx[b]
                    kk = k % 4
                    osb = osbx[(b, k // 4)]
                    e_k = E[:, k * 128 : (k + 1) * 128]
                    rr = rrx[(b, k)]
                    psB = ps_ab.tile([128, D], fp32, tag="ab", name=f"psB{b}_{k}")
                    nc.tensor.matmul(psB, e_k, T1s, start=True, stop=True)
                    # CBm = (psB * rr) * Cb (fused STT; DVE reads PSUM)
                    nc.vector.scalar_tensor_tensor(
                        osb[:, kk, 2 * D : 3 * D], psB, rr, Cb1[:, k, 0:D],
                        MULT, MULT,
                    )
                    if b == NB - 1:
                        # last batch: store pairs immediately on 3 rings to
                        # shrink the final DMA drain
                        if kk % 2 == 1:
                            eng2 = (nc.sync, nc.gpsimd, nc.scalar, nc.sync)[k // 2]
                            eng2.dma_start(
                                out_d.ap()[
                                    b, (k - 1) * 128 : (k + 1) * 128, :
                                ].rearrange("(k p) n -> p k n", p=128),
                                osb[:, kk - 1 : kk + 1],
                            )
                    elif kk == 3:
                        eng2 = nc.sync if (2 * b + k // 4) % 2 == 0 else nc.gpsimd
                        eng2.dma_start(
                            out_d.ap()[
                                b, (k - 3) * 128 : (k + 1) * 128, :
                            ].rearrange("(k p) n -> p k n", p=128),
                            osb,
                        )

                return [(lambda kk_: lambda: tile_b(kk_))(k) for k in range(KC)]

            # fine-grained software pipelining: interleave head(b) stages
            # with tail(b-1) stages so no engine queue head-of-line blocks
            for step in range(NB + 1):
                hs = head_stages(step) if step < NB else []
                ts = tail_stages(step - 1) if step >= 1 else []
                n = max(len(hs), len(ts))
                for i in range(n):
                    if i < len(hs):
                        hs[i]()
                    if i < len(ts):
                        ts[i]()

    nc.compile()
    return nc


def _get_nc():
    if "nc" not in _CACHE:
        _CACHE["nc"] = _build_nc()
    return _CACHE["nc"]


def _make_in_maps(C, Q, cmask, qmask, Wo_w):
    import ml_dtypes

    bf16 = ml_dtypes.bfloat16
    C = np.ascontiguousarray(C, dtype=np.float32)
    Q = np.ascontiguousarray(Q, dtype=np.float32)
    Wo_w = Wo_w.astype(np.float32)
    w1, w2, w3 = Wo_w[:D], Wo_w[D : 2 * D], Wo_w[2 * D :]

    NBND = KC * (D + 2) + 2 * 128 + (D + 1) + 3
    O_QT = KC * (D + 2)
    O_QB = O_QT + 2 * 128
    O_RQ = O_QB + (D + 1) + 1
    bnd = np.zeros((B_FULL, 128, NBND), dtype=bf16)

    # Cb: tile layout with a ones column (t1 colsum source)
    cb = bnd[:, :, 0:O_QT].reshape(B_FULL, 128, KC, D + 2)
    cb[:, :, :, 0:D] = C.reshape(B_FULL, KC, 128, D).transpose(0, 2, 1, 3)
    cb[:, :, :, D] = 1.0

    # QT3: [p, dk, j<128] = Q[b,j,dk*128+p]*w3[dk*128+p]
    qt3 = bnd[:, :, O_QT:O_QB].reshape(B_FULL, 128, 2, 128)
    qt = Q.transpose(0, 2, 1).reshape(B_FULL, 2, 128, 128).transpose(0, 2, 1, 3)
    qt3[:] = qt * w3.reshape(2, 128).T[None, :, :, None]

    # Qb = [Q | 1] (rowsum source)
    bnd[:, :, O_QB : O_QB + D] = Q
    bnd[:, :, O_QB + D] = 1.0

    # rq + finite qneg exp bias
    bnd[:, :, O_RQ] = Q @ w2 + (1.0 - qmask.astype(np.float32)) * MASK_NEG

    # rc + finite cneg rows (rank-1 matmul operand)
    rc = C @ w1 + (1.0 - cmask.astype(np.float32)) * MASK_NEG  # [B, Lc]
    rc2 = rc.astype(bf16)

    in_maps = []
    for i in range(N_CORES):
        sl = slice(i * NB, (i + 1) * NB)
        in_maps.append(
            {
                "bnd": np.ascontiguousarray(bnd[sl]),
                "rc": np.ascontiguousarray(rc2[sl].reshape(1, NB * 2 * 512)),
            }
        )
    return in_maps


def kernel(C, Q, cmask, qmask, Wo_w, Wo_b):
    from concourse.bass_utils import run_bass_kernel_spmd

    nc = _get_nc()
    in_maps = _make_in_maps(C, Q, cmask, qmask, Wo_w)
    res = run_bass_kernel_spmd(nc, in_maps, core_ids=list(range(N_CORES)))
    dev = np.concatenate(
        [res.results[i]["out"] for i in range(N_CORES)], axis=0
    )  # [B, Lc, 3d] bf16
    out = np.empty((B_FULL, LC, 4 * D), dtype=np.float32)
    out[:, :, 0:D] = C
    out[:, :, D:] = dev.astype(np.float32)
    return out
